# revision 14
# baseline (speedup 1.0000x reference)
"""Linear attention kernel for 8 NeuronCores (Trainium2, Bass/Tile).

Strategy
--------
Shard over (batch b, head-group hg): core c -> b = c//2, hg = c%2 (2 heads of
d=64 each, 128 "hd" columns per core).  Host pre-transposes x[b] -> xT [D, T]
so the contraction dim D lands on SBUF partitions, and slices/transposes the
weights per head-group.

The scan recurrence is algebraically chunked causal linear attention: per
128-step chunk, score S^T = (k-chunk)^T q-chunk (d-major operands), masked
lower-triangular, num/den computed via matmuls into one [t, 65] PSUM tile
(cols 0-63 numerator, col 64 denominator) accumulating the inter-chunk part
from the running KV state.  State (KV [d, j] + k-sum col) accumulates in one
persistent PSUM tile across all 32 chunks.

q's positive-feature normalization cancels in num/den (up to EPS scaling,
rel err ~1e-8) so it is skipped; k is normalized in time-major layout after
the per-chunk PE transpose, where the L2 norm over d is a free-dim reduction.
"""
import sys

sys.path.insert(0, "/opt/trn_rl_repo")

import numpy as np

B, T, D = 4, 4096, 1024
H, DH = 4, 64
HD = 128          # two heads per core
TB = 512          # time block
NB = T // TB      # 8 blocks
CH = 128          # recurrence chunk
NCH = TB // CH    # 4 chunks per block
NCHT = T // CH    # 32 chunks total
EPS = 1e-6
NCORES = 8

USE_F32R = True  # fast fp32 streaming mode for the big projection matmuls
DEBUG = False

_cache = {}


def _build_program():
    import concourse.bacc as bacc
    import concourse.tile as tile
    from concourse import mybir
    from contextlib import ExitStack

    f32 = mybir.dt.float32
    f32r = mybir.dt.float32r
    AF = mybir.ActivationFunctionType
    OP = mybir.AluOpType

    def mmcast(ap):
        return ap.bitcast(f32r) if USE_F32R else ap

    nc = bacc.Bacc("TRN2", target_bir_lowering=False, debug=False,
                   num_devices=NCORES)

    fin = f32r if USE_F32R else f32
    xT = nc.dram_tensor("xT", [D, T], fin, kind="ExternalInput").ap()
    wqT = nc.dram_tensor("wqT", [D, HD], fin, kind="ExternalInput").ap()
    wkT = nc.dram_tensor("wkT", [D, HD], fin, kind="ExternalInput").ap()
    wvT = nc.dram_tensor("wvT", [D, HD], fin, kind="ExternalInput").ap()
    woT = nc.dram_tensor("woT", [HD, D], fin, kind="ExternalInput").ap()
    maskI = nc.dram_tensor("mask", [CH, CH], f32, kind="ExternalInput").ap()
    identI = nc.dram_tensor("ident", [128, 128], f32, kind="ExternalInput").ap()
    onesI = nc.dram_tensor("ones", [128, 1], f32, kind="ExternalInput").ap()
    yT = nc.dram_tensor("yT", [D, T], f32, kind="ExternalOutput").ap()
    stO = nc.dram_tensor("state", [HD, DH + 1], f32, kind="ExternalOutput").ap()
    if DEBUG:
        dbgS = nc.dram_tensor("dbgS", [NCHT, 128, DH + 1], f32,
                              kind="ExternalOutput").ap()
        dbgV = nc.dram_tensor("dbgV", [NCHT, 128, 128], f32,
                              kind="ExternalOutput").ap()
        dbgK = nc.dram_tensor("dbgK", [NCHT, 128, 128], f32,
                              kind="ExternalOutput").ap()

    with tile.TileContext(nc) as tc, ExitStack() as ctx:
        cpool = ctx.enter_context(tc.tile_pool(name="const", bufs=1))
        xpool = ctx.enter_context(tc.tile_pool(name="xt", bufs=16))
        bpool = ctx.enter_context(tc.tile_pool(name="big", bufs=2))
        ypool = ctx.enter_context(tc.tile_pool(name="yout", bufs=3))
        tpool = ctx.enter_context(tc.tile_pool(name="tmp", bufs=2))
        chpool = ctx.enter_context(tc.tile_pool(name="chunk", bufs=8))
        spool = ctx.enter_context(tc.tile_pool(name="small", bufs=6))
        stpool = ctx.enter_context(tc.tile_pool(name="stt", bufs=1))
        pp = ctx.enter_context(tc.tile_pool(name="pp", bufs=2, space="PSUM"))
        pa = ctx.enter_context(tc.tile_pool(name="pa", bufs=5, space="PSUM"))
        pst = ctx.enter_context(tc.tile_pool(name="pst", bufs=1, space="PSUM"))

        # constants
        wq_sb = cpool.tile([128, 8, 128], fin, tag="wq")
        wk_sb = cpool.tile([128, 8, 128], fin, tag="wk")
        wv_sb = cpool.tile([128, 8, 128], fin, tag="wv")
        wo_sb = cpool.tile([128, 1024], fin, tag="wo")
        mask = cpool.tile([CH, CH], f32, tag="mask")
        ident = cpool.tile([128, 128], f32, tag="ident")
        ones = cpool.tile([128, 1], f32, tag="ones")
        nc.sync.dma_start(wq_sb[:], wqT.rearrange("(n p) m -> p n m", p=128))
        nc.sync.dma_start(wk_sb[:], wkT.rearrange("(n p) m -> p n m", p=128))
        nc.sync.dma_start(wv_sb[:], wvT.rearrange("(n p) m -> p n m", p=128))
        nc.sync.dma_start(wo_sb[:], woT[:])
        nc.sync.dma_start(mask[:], maskI[:])
        nc.sync.dma_start(ident[:], identI[:])
        nc.sync.dma_start(ones[:], onesI[:])

        # persistent state: KV matrix and k-sum column live in SEPARATE
        # PSUM banks -- a start=True write to one column-region of a bank
        # clears has_written for the whole bank row, breaking any other
        # accumulation region in the same bank.
        kv_ps = pst.tile([128, DH + 1], f32, tag="kvps")  # [d(+64*h), j | ksum]
        kv_sb = stpool.tile([128, DH + 1], f32, tag="kvsb")

        for b in range(NB):
            t0b = b * TB
            xts = []
            for dt in range(8):
                xt_t = xpool.tile([128, TB], fin, tag="xt")
                nc.sync.dma_start(
                    xt_t[:], xT[dt * 128:(dt + 1) * 128, t0b:t0b + TB])
                xts.append(xt_t)

            # projections (d-major outputs [hd, t])
            def project(w_sb, tag):
                ps = pp.tile([128, TB], f32, tag="pp")
                for dt in range(8):
                    nc.tensor.matmul(ps[:], w_sb[:, dt, :], xts[dt][:],
                                     start=(dt == 0), stop=(dt == 7))
                return ps

            qp = project(wq_sb, "q")
            # elu(x)+1 = exp(min(x,0)) + relu(x); q stays unnormalized
            rq = tpool.tile([128, TB], f32, tag="t0")
            nc.scalar.activation(rq[:], qp[:], AF.Relu)
            mq = tpool.tile([128, TB], f32, tag="t1")
            nc.vector.tensor_scalar_min(mq[:], qp[:], 0.0)
            eq = tpool.tile([128, TB], f32, tag="t2")
            nc.scalar.activation(eq[:], mq[:], AF.Exp)
            fq = bpool.tile([128, TB], f32, tag="fq")
            nc.vector.tensor_tensor(fq[:], eq[:], rq[:], OP.add)

            kp = project(wk_sb, "k")
            rk = tpool.tile([128, TB], f32, tag="t0")
            nc.scalar.activation(rk[:], kp[:], AF.Relu)
            mk = tpool.tile([128, TB], f32, tag="t1")
            nc.vector.tensor_scalar_min(mk[:], kp[:], 0.0)
            ek = tpool.tile([128, TB], f32, tag="t2")
            nc.scalar.activation(ek[:], mk[:], AF.Exp)
            fk = bpool.tile([128, TB], f32, tag="fk")
            nc.vector.tensor_tensor(fk[:], ek[:], rk[:], OP.add)

            vp = project(wv_sb, "v")
            vsb = bpool.tile([128, TB], f32, tag="vsb")
            nc.scalar.copy(vsb[:], vp[:])

            aoT = bpool.tile([128, TB], fin, tag="aot")

            # pass 1: per-chunk work that does not depend on the running
            # state -- k/v transposes, norm weights, scores + masking.  Emitted
            # for all chunks first so the in-order engine queues pipeline
            # across chunks instead of ping-ponging through one chunk's chain.
            ktrs, vtmss, stms = [], [], []
            for c in range(NCH):
                t0 = c * CH
                ktp = pa.tile([128, 128], f32, tag="pa", name=f"ktp{c}")
                nc.tensor.transpose(ktp[:], fk[:, t0:t0 + CH], ident[:])
                ktr = chpool.tile([128, 128], f32, tag="ktr", name=f"ktr{c}")
                nc.scalar.copy(ktr[:], ktp[:])
                sqt = chpool.tile([128, 128], f32, tag="ttrs", name=f"sq{c}")
                nc.scalar.activation(sqt[:], ktp[:], AF.Square)
                ssq = spool.tile([128, 2], f32, tag="ssq", name=f"ssq{c}")
                nc.vector.tensor_reduce(
                    ssq[:], sqt[:].rearrange("p (g d) -> p g d", g=2),
                    mybir.AxisListType.X, OP.add)
                sr = spool.tile([128, 2], f32, tag="sr", name=f"sr{c}")
                nc.scalar.activation(sr[:], ssq[:], AF.Sqrt)
                sre = spool.tile([128, 2], f32, tag="sre", name=f"sre{c}")
                nc.vector.tensor_scalar_add(sre[:], sr[:], EPS)
                w = spool.tile([128, 2], f32, tag="rn", name=f"w{c}")
                nc.vector.reciprocal(w[:], sre[:])
                vtp = pa.tile([128, 128], f32, tag="pa", name=f"vtp{c}")
                nc.tensor.transpose(vtp[:], vsb[:, t0:t0 + CH], ident[:])
                vtm0 = chpool.tile([128, DH + 1], f32, tag="vtm0",
                                   name=f"vtm0_{c}")
                nc.scalar.mul(vtm0[:, 0:DH], vtp[:, 0:64], w[:, 0:1])
                nc.vector.tensor_copy(vtm0[:, DH:DH + 1], w[:, 0:1])
                vtm1 = chpool.tile([128, DH + 1], f32, tag="vtm1",
                                   name=f"vtm1_{c}")
                nc.scalar.mul(vtm1[:, 0:DH], vtp[:, 64:128], w[:, 1:2])
                nc.vector.tensor_copy(vtm1[:, DH:DH + 1], w[:, 1:2])
                stm_c = []
                for h in range(2):
                    hs = slice(64 * h, 64 * h + 64)
                    st_ps = pa.tile([128, 128], f32, tag="pa",
                                    name=f"st{h}_{c}")
                    nc.tensor.matmul(st_ps[:], fk[hs, t0:t0 + CH],
                                     fq[hs, t0:t0 + CH],
                                     start=True, stop=True)
                    stm = chpool.tile([128, 128], f32, tag=f"stm{h}",
                                      name=f"stm{h}_{c}")
                    nc.vector.tensor_tensor(stm[:], st_ps[:], mask[:], OP.mult)
                    stm_c.append(stm)
                ktrs.append(ktr)
                vtmss.append((vtm0, vtm1))
                stms.append(stm_c)

            # pass 2: the state recurrence + output assembly
            for c in range(NCH):
                g = b * NCH + c
                t0 = c * CH
                ktr = ktrs[c]
                vtms = vtmss[c]
                ao = chpool.tile([128, 128], f32, tag="ao", name=f"ao{c}")
                for h in range(2):
                    hs = slice(64 * h, 64 * h + 64)
                    stm = stms[c][h]
                    nd = pa.tile([128, DH + 1], f32, tag="pa",
                                 name=f"nd{h}_{c}")
                    if g > 0:
                        nc.tensor.matmul(nd[:, 0:DH + 1],
                                         fq[hs, t0:t0 + CH],
                                         kv_sb[hs.start:hs.start + 64, :],
                                         start=True, stop=False,
                                         skip_group_check=True)
                    nc.tensor.matmul(nd[:, 0:DH + 1], stm[:],
                                     vtms[h][:, 0:DH + 1],
                                     start=(g == 0), stop=True,
                                     skip_group_check=True)
                    de = spool.tile([128, 1], f32, tag=f"de{h}",
                                    name=f"de{h}_{c}")
                    nc.vector.tensor_scalar_add(de[:], nd[:, DH:DH + 1], EPS)
                    rc = spool.tile([128, 1], f32, tag=f"rc{h}",
                                    name=f"rc{h}_{c}")
                    nc.vector.reciprocal(rc[:], de[:])
                    nc.scalar.mul(ao[:, hs], nd[:, 0:DH], rc[:])
                    # state update (h1 lands on PSUM partitions 64-127 via
                    # col-group packing)
                    ob = 64 * h
                    nc.tensor.matmul(kv_ps[ob:ob + 64, 0:DH + 1], ktr[:, hs],
                                     vtms[h][:, 0:DH + 1], start=(g == 0),
                                     stop=(g == NCHT - 1),
                                     skip_group_check=True)
                nc.scalar.copy(kv_sb[:], kv_ps[:])
                # ao -> d-major, append to block output
                aop = pa.tile([128, 128], f32, tag="pa", name=f"aop{c}")
                nc.tensor.transpose(aop[:], ao[:], ident[:])
                nc.vector.tensor_copy(aoT[:, t0:t0 + CH], aop[:])

            # output projection
            for dt in range(8):
                yp = pp.tile([128, TB], f32, tag="pp")
                nc.tensor.matmul(yp[:], wo_sb[:, dt * 128:(dt + 1) * 128],
                                 aoT[:], start=True, stop=True)
                ysb = ypool.tile([128, TB], f32, tag="ysb")
                if dt % 2 == 0:
                    nc.vector.tensor_copy(ysb[:], yp[:])
                else:
                    nc.scalar.copy(ysb[:], yp[:])
                nc.sync.dma_start(
                    yT[dt * 128:(dt + 1) * 128, t0b:t0b + TB], ysb[:])

        nc.sync.dma_start(stO[:], kv_sb[:])

    nc.compile()
    return nc


def _get_program():
    if "nc" not in _cache:
        _cache["nc"] = _build_program()
    return _cache["nc"]


def kernel(x, Wq, Wk, Wv, Wo):
    from concourse.bass_utils import run_bass_kernel_spmd

    nc = _get_program()

    mask_np = np.triu(np.ones((CH, CH), dtype=np.float32))
    ident_np = np.eye(128, dtype=np.float32)
    ones_np = np.ones((128, 1), dtype=np.float32)

    in_maps = []
    for c in range(NCORES):
        b, hg = c // 2, c % 2
        hsl = slice(hg * HD, (hg + 1) * HD)
        in_maps.append({
            "xT": np.ascontiguousarray(x[b].T),
            "wqT": np.ascontiguousarray(Wq[hsl, :].T),
            "wkT": np.ascontiguousarray(Wk[hsl, :].T),
            "wvT": np.ascontiguousarray(Wv[hsl, :].T),
            "woT": np.ascontiguousarray(Wo[:, hsl].T),
            "mask": mask_np,
            "ident": ident_np,
            "ones": ones_np,
        })

    res = run_bass_kernel_spmd(nc, in_maps, list(range(NCORES)))

    y = np.zeros((B, T, D), dtype=np.float32)
    kv_f = np.zeros((B, H, DH, DH), dtype=np.float32)
    kc_f = np.zeros((B, H, DH), dtype=np.float32)
    for c in range(NCORES):
        b, hg = c // 2, c % 2
        out = res.results[c]
        y[b] += out["yT"].T
        st = out["state"]
        for hl in range(2):
            hglob = hg * 2 + hl
            blk = st[hl * 64:(hl + 1) * 64, :]
            kv_f[b, hglob] = blk[:, 0:DH].T   # KV[d, j] -> kv_cum[j, d]
            kc_f[b, hglob] = blk[:, DH]
    return y, kv_f, kc_f
